# revision 12
# baseline (speedup 1.0000x reference)
"""Trainium2 Bass kernel for CompoundEmbedding (embedding-bag sum).

Problem: indices (16384, 50) -> gather rows of weight (100001, 128) f32,
sum over the bag dim -> output (16384, 128) f32.

Strategy (v3): the only fast data-dependent move on TRN2 is SWDGE
descriptor generation, and `dma_gather` (ext-isa, int16 indices, <=1024
idxs/instruction, 4 SWDGE queues on disjoint Q7 core pairs) generates at
~3.1ns/row when instructions rotate over the 4 queues — ~3x the
single-queue rate. int16 can only address 32768 rows, so the table is
host-packed into w4 [25001, 512] fp16: 4 "colors" of <=25001 vocab rows,
color r at columns [r*128,(r+1)*128); a gather for color r uses
elem_step=512 (1024B stride) and base offset r*256B, with q = slot index
<= 25000 (fits int16). The host 4-colors the vocab (iterative rebalance)
so every output row has <=16 lookups per color, then pads each (row,
color) list to exactly 16 slots with a per-color all-zero row. Per core
(2048 output rows = 16 blocks of 128):
  - per block: 8 dma_gathers (4 colors x 2 halves, 1024 idxs each,
    queue = instr mod 4) fill gt [128 partitions, 64 slots, 128 d] fp16,
    partition p = output row p of the block, in list order
    position k -> (partition k%128, slot k//128);
  - DVE pairwise tree (6 adds, last two levels f32) -> [128, 128] f32;
  - store the block to DRAM.
Indices are uploaded pre-swizzled ([16, 64] wrap replicated x8 per
instruction, int16). fp16 keeps rel err ~1e-3 << 2e-2 tolerance.
All shapes/sharding are hardcoded for this problem instance.
"""

import numpy as np

NUM_EMB = 100001
D = 128
B = 16384
BAG = 50
NCORES = 8
P = 128
ROWS_PER_CORE = B // NCORES  # 2048
NBLK = ROWS_PER_CORE // P  # 16

NCOLOR = 4
TSLOT = 16           # slots per (row, color)
SHALF = TSLOT // 4   # slots per gather instruction
NHALF = TSLOT // SHALF
NIDX = SHALF * P     # idxs per gather (<= 1024: SWDGE ring cap ~65 descs)
VQ = 25600           # super-rows in the packed table (slack over 100001/4
                     # so the coloring needs no per-color capacity balancing;
                     # q fits int16 easily)
NINSTR = NCOLOR * NHALF  # gathers per block
IDXW = NINSTR * (NIDX // 16)  # int16 columns per block in the idx tensor

_CACHE = {}


def _build(nblk=NBLK, loop_k=1):
    import contextlib
    import concourse.bass as bass
    import concourse.tile as tile
    from concourse import bacc, mybir
    from concourse.library_config import mlp

    rows = nblk * P
    nc = bacc.Bacc("TRN2", target_bir_lowering=False, debug=False,
                   num_devices=NCORES, num_swdge_queues=4)
    idx_d = nc.dram_tensor("idxq", [P, nblk * IDXW], mybir.dt.int16,
                           kind="ExternalInput").ap()
    w_d = nc.dram_tensor("w4", [VQ, NCOLOR * D], mybir.dt.float16,
                         kind="ExternalInput").ap()
    out_d = nc.dram_tensor("out", [rows, D], mybir.dt.float32,
                           kind="ExternalOutput").ap()

    qcounter = [0]

    with tile.TileContext(nc) as tc:
        with tc.tile_pool(name="idxp", bufs=3) as idxp, \
             tc.tile_pool(name="gat", bufs=3) as gatp, \
             tc.tile_pool(name="red", bufs=2) as redp:
            nc.gpsimd.load_library(mlp)
            loop_stack = contextlib.ExitStack()
            if loop_k > 1:
                loop_stack.enter_context(tc.For_i(0, loop_k, 1))
                nc.tensor.nop()
                nc.scalar.nop()
            for blk in range(nblk):
                it = idxp.tile([P, IDXW], mybir.dt.int16)
                nc.sync.dma_start(
                    out=it[:], in_=idx_d[:, blk * IDXW:(blk + 1) * IDXW])
                gt = gatp.tile([P, NINSTR * SHALF * D], mybir.dt.float16)
                for r in range(NCOLOR):
                    for h in range(NHALF):
                        j = r * NHALF + h
                        q = qcounter[0] % 4
                        qcounter[0] += 1
                        nc.gpsimd.dma_gather(
                            out_ap=gt[:, j * SHALF * D:(j + 1) * SHALF * D]
                                .rearrange("p (s d) -> p s d", s=SHALF),
                            in_ap=w_d[:, r * D:(r + 1) * D],
                            idxs_ap=it[:, j * (NIDX // 16):
                                       (j + 1) * (NIDX // 16)],
                            num_idxs=NIDX,
                            num_idxs_reg=NIDX,
                            elem_size=D,
                            elem_step=NCOLOR * D,
                            transpose=False,
                            queue_num=q,
                        )
                # pairwise tree over 64 slots; first level as 4 pair-adds so
                # DVE starts as soon as two gathers have landed (sum is
                # order-independent)
                r32 = redp.tile([P, 32 * D], mybir.dt.float16)
                for j in range(NINSTR // 2):
                    nc.vector.tensor_add(
                        r32[:, j * SHALF * D:(j + 1) * SHALF * D],
                        gt[:, (2 * j) * SHALF * D:(2 * j + 1) * SHALF * D],
                        gt[:, (2 * j + 1) * SHALF * D:(2 * j + 2) * SHALF * D])
                r16 = redp.tile([P, 16 * D], mybir.dt.float16)
                nc.vector.tensor_add(r16[:], r32[:, 0:16 * D],
                                     r32[:, 16 * D:32 * D])
                r8 = redp.tile([P, 8 * D], mybir.dt.float16)
                nc.vector.tensor_add(r8[:], r16[:, 0:8 * D],
                                     r16[:, 8 * D:16 * D])
                r4 = redp.tile([P, 4 * D], mybir.dt.float16)
                nc.vector.tensor_add(r4[:], r8[:, 0:4 * D], r8[:, 4 * D:8 * D])
                r2 = redp.tile([P, 2 * D], mybir.dt.float32)
                nc.vector.tensor_add(r2[:], r4[:, 0:2 * D], r4[:, 2 * D:4 * D])
                rf = redp.tile([P, D], mybir.dt.float32)
                nc.vector.tensor_add(rf[:], r2[:, 0:D], r2[:, D:2 * D])
                # store via ACT's HWDGE so the SP stream stays a pure
                # idx-load prefetch queue (stores wait on DVE; loads must not
                # sit behind them on the same in-order sequencer)
                nc.scalar.dma_start(out=out_d[blk * P:(blk + 1) * P, :],
                                    in_=rf[:])
            loop_stack.close()
    nc.compile()
    return nc


def _get_program(nblk=NBLK, loop_k=1):
    key = (nblk, loop_k)
    if key not in _CACHE:
        _CACHE[key] = _build(nblk, loop_k)
    return _CACHE[key]


def _color_vocab(idx):
    """4-color the vocab so each output row has <= TSLOT lookups per color.
    VQ has enough slack that per-color capacity needs no balancing."""
    rng = np.random.default_rng(1234)
    rows = np.repeat(np.arange(B), BAG)
    vs = idx.ravel()
    color = (np.arange(NUM_EMB) % NCOLOR).astype(np.int8)

    def counts_of(col):
        cnt = np.zeros((B, NCOLOR), dtype=np.int32)
        np.add.at(cnt, (rows, col[vs]), 1)
        return cnt

    for _ in range(400):
        cnt = counts_of(color)
        bad = np.where(cnt.max(axis=1) > TSLOT)[0]
        if bad.size == 0:
            break
        amax = cnt[bad].argmax(axis=1).astype(np.int8)
        amin = cnt[bad].argmin(axis=1).astype(np.int8)
        lc = color[idx[bad]]
        hit = lc == amax[:, None]
        pri = rng.random((bad.size, BAG)) * hit
        pick = pri.argmax(axis=1)
        vsel = idx[bad, pick]
        color[vsel] = amin
    else:
        raise RuntimeError("vocab coloring did not converge")

    n_c = np.bincount(color, minlength=NCOLOR)
    assert (n_c <= VQ - 1).all(), n_c
    return color


def prepare_inputs(input, weight):
    """Host preprocessing: coloring, packed fp16 table, swizzled int16
    index tensors. Returns in_maps for run_bass_kernel_spmd."""
    idx = np.asarray(input).astype(np.int64)
    w = np.asarray(weight, dtype=np.float32)
    assert idx.shape == (B, BAG) and w.shape == (NUM_EMB, D)

    color = _color_vocab(idx)

    # q assignment: within each color, number rows 0..n_c-1
    q_of = np.zeros(NUM_EMB, dtype=np.int32)
    vlists = []
    for c in range(NCOLOR):
        vl = np.where(color == c)[0]
        q_of[vl] = np.arange(vl.size)
        vlists.append(vl)

    # packed table [VQ, 4*D] fp16; unassigned slots stay zero
    w4 = np.zeros((VQ, NCOLOR * D), dtype=np.float16)
    for c in range(NCOLOR):
        vl = vlists[c]
        w4[:vl.size, c * D:(c + 1) * D] = w[vl].astype(np.float16)

    # per-color zero-pad q: first unassigned slot (zero-filled)
    pad_q = np.array([vlists[c].size for c in range(NCOLOR)], dtype=np.int32)

    # slot table: [B, NCOLOR, TSLOT] of q values (pad-filled)
    qtab = np.tile(pad_q[None, :, None], (B, 1, TSLOT)).astype(np.int16)
    lc = color[idx]                      # [B, BAG] color of each lookup
    lq = q_of[idx]                       # [B, BAG] q of each lookup
    order = np.lexsort((np.tile(np.arange(BAG), (B, 1)).ravel(),
                        lc.ravel(),
                        np.repeat(np.arange(B), BAG)))
    rs = np.repeat(np.arange(B), BAG)[order]
    cs = lc.ravel()[order]
    qs = lq.ravel()[order]
    # position within (row, color) group
    grp = rs * NCOLOR + cs
    first = np.r_[True, grp[1:] != grp[:-1]]
    gidx = np.arange(grp.size)
    start = np.maximum.accumulate(np.where(first, gidx, 0))
    slot = gidx - start
    assert slot.max() < TSLOT
    qtab[rs, cs, slot] = qs.astype(np.int16)

    # per-core idx tensors: [P, NBLK * IDXW] int16
    # per (core, block, instr j=(c,h)): A[p, s] = qtab[row, c, h*SHALF+s]
    # tile[j2, s*8+g] = A[16g+j2, s]; replicate x8 over partition groups
    qtab_c = qtab.reshape(NCORES, NBLK, P, NCOLOR, NHALF, SHALF)
    in_maps = []
    idxall = np.empty((NCORES, P, NBLK * IDXW), dtype=np.int16)
    for core in range(NCORES):
        blocks = []
        for blk in range(NBLK):
            instrs = []
            for c in range(NCOLOR):
                for h in range(NHALF):
                    A = qtab_c[core, blk, :, c, h, :]       # [128, SHALF]
                    T = A.reshape(8, 16, SHALF).transpose(1, 2, 0) \
                         .reshape(16, SHALF * 8)            # [16, 64]
                    instrs.append(np.tile(T, (8, 1)))       # [128, 64]
            blocks.append(np.concatenate(instrs, axis=1))   # [128, IDXW]
        idxall[core] = np.concatenate(blocks, axis=1)
        in_maps.append({"idxq": idxall[core], "w4": w4})
    return in_maps


def kernel(input, weight):
    from concourse.bass_utils import run_bass_kernel_spmd

    in_maps = prepare_inputs(input, weight)
    nc = _get_program()
    res = run_bass_kernel_spmd(nc, in_maps, core_ids=list(range(NCORES)))
    out = np.concatenate([res.results[c]["out"] for c in range(NCORES)],
                         axis=0)
    return out
